# revision 5
# baseline (speedup 1.0000x reference)
import sys

if "/opt/trn_rl_repo" not in sys.path:
    sys.path.insert(0, "/opt/trn_rl_repo")

import numpy as np
import ml_dtypes

import concourse.bass as bass
import concourse.bacc as bacc
import concourse.tile as tile
import concourse.mybir as mybir
from concourse import bass_utils

# Problem shapes (nn_ChebConv): x (16, 12288), L (12288, 12288),
# weights (5, 16, 32), bias (32,). out (32, 12288).
#
# Sharding: core d owns V-columns [d*1536, (d+1)*1536).  Host feeds each
# core lt = L^T[:, cols_d] (contraction dim on partitions), row-interleaved
# within 512-row groups so each SBUF partition reads one contiguous chunk
# per bulk DMA.
#
# Schedule: step 1 streams ALL 96 vc-tiles (37.7 MB) at full HBM rate on
# two parallel queues (streamed tiles -> ltp pool on sync, resident tiles
# -> rs_sb on vector), with the PE chasing the stream; steps 2-4 touch
# only the 52 streamed tiles (resident 44 stay in SBUF), making them
# PE-bound (~62us each).  Step boundary: per-chunk DVE + PE-transpose +
# stage, one small AllGather (warmed to ~7us by 4 warm-up AGs), stationary
# reload.  Tail: one fused einsum matmul per 512-chunk contracting all of
# T_0..T_3 at once (w stacked on 128 partitions) + T_4 accumulate + bias.
C_IN = 16
C_OUT = 32
K_CHEB = 5
V = 12288
N_CORES = 8
VLOC = V // N_CORES          # 1536 columns of the V axis per core
P = 128
NT_VC = V // P               # 96 contraction tiles per step
N_CH = VLOC // 512           # 3 psum chunks of 512
NB = 4                       # vc-tiles per bulk lt DMA (512 rows)
NG = NT_VC // NB             # 24 groups
RES_T = 44                   # vc-tiles resident in SBUF (of 96)
RES_G = RES_T // NB          # 11 resident groups
LT_BUFS = 3
NB_S = VLOC // P             # stationary group: 12 vc-tiles = one rank chunk
NG_S = V // (P * NB_S)       # 8 stationary groups

_CACHE: dict = {}


def _build(cfg: str):
    if cfg == "bf16":
        mm_dt = mybir.dt.bfloat16
    else:
        mm_dt = mybir.dt.float32
    f32 = mybir.dt.float32

    res_g = RES_G

    nc = bacc.Bacc("TRN2", target_bir_lowering=False, debug=False,
                   num_devices=N_CORES)

    lt = nc.dram_tensor("lt", [V, VLOC], mm_dt, kind="ExternalInput")
    xt = nc.dram_tensor("xt", [V, C_IN], mm_dt, kind="ExternalInput")
    xc = nc.dram_tensor("xc", [C_IN, VLOC], f32, kind="ExternalInput")
    wf = nc.dram_tensor("wf", [P, C_OUT], f32, kind="ExternalInput")
    w4 = nc.dram_tensor("w4", [C_IN, C_OUT], f32, kind="ExternalInput")
    bias_in = nc.dram_tensor("bias_in", [C_OUT, 1], f32, kind="ExternalInput")
    id128 = nc.dram_tensor("id128", [P, C_IN], f32, kind="ExternalInput")
    out = nc.dram_tensor("out", [C_OUT, VLOC], f32, kind="ExternalOutput")

    lt_r = lt.ap().rearrange("(g p u) c -> g p u c", p=P, u=NB)

    with tile.TileContext(nc) as tc:
        with (
            tc.tile_pool(name="ltp", bufs=LT_BUFS) as ltp,
            tc.tile_pool(name="persist", bufs=1) as persist,
            tc.tile_pool(name="stat", bufs=2 * NG_S) as statp,
            tc.tile_pool(name="work", bufs=2) as work,
            tc.tile_pool(name="acc", bufs=4, space="PSUM") as accp,
            tc.tile_pool(name="tpp", bufs=4, space="PSUM") as tpp,
            tc.tile_pool(name="dram", bufs=1, space="DRAM") as dram,
        ):
            # ---- persistent small tensors ----
            w_sb = persist.tile([P, C_OUT], f32)
            nc.scalar.dma_start(w_sb[:], wf.ap())
            w4_sb = persist.tile([C_IN, C_OUT], f32)
            nc.scalar.dma_start(w4_sb[:], w4.ap())
            bias_sb = persist.tile([C_OUT, 1], f32)
            nc.scalar.dma_start(bias_sb[:], bias_in.ap())
            id_sb = persist.tile([P, C_IN], f32)
            nc.scalar.dma_start(id_sb[:], id128.ap())

            # T_0..T_3 stacked at partition bases {0,32,64,96} of one tile
            # (rows 16-31 of each block must be ZERO: the fused einsum
            # contracts all 128 partitions against zero-padded weights).
            t_blk = persist.tile([P, VLOC], f32)
            t4_sb = persist.tile([C_IN, VLOC], f32)
            nc.vector.memset(t_blk[:], 0.0)
            nc.scalar.dma_start(t_blk[0:C_IN, :], xc.ap())

            def t_ap(k):
                if k == K_CHEB - 1:
                    return t4_sb[:]
                return t_blk[32 * k:32 * k + C_IN, :]

            # stationary tensors (x^T, then each gathered T_k^T) live in
            # rank-aligned 1536-row groups, row-interleaved so partition p
            # reads rows [g*1536 + 12p, +12) — one 384 B chunk.
            def load_stat(src_r, tag_k):
                tiles = []
                for g in range(NG_S):
                    s = statp.tile([P, NB_S * C_IN], mm_dt,
                                   name=f"st{tag_k}_{g}", tag="stat")
                    nc.scalar.dma_start(
                        s[:].rearrange("p (j c) -> p j c", j=NB_S), src_r[g])
                    tiles.append(s)
                return tiles

            xt_r = xt.ap().rearrange("(g p j) c -> g p j c", p=P, j=NB_S)
            sk_tiles = load_stat(xt_r, 0)

            # tiny warm-up AllGathers: pay the ~70us first-collective cost
            # while step 1 streams, not on the critical path
            wu_sb = work.tile([P, C_IN], mm_dt, name="wu_sb", tag="wu")
            nc.vector.memset(wu_sb[:], 0.0)
            for w in range(2):
                wu_in = dram.tile([P, C_IN], mm_dt, name=f"wu_in{w}")
                wu_out = dram.tile([P * N_CORES, C_IN], mm_dt,
                                   name=f"wu_out{w}")
                nc.scalar.dma_start(wu_in[:], wu_sb[:])
                nc.gpsimd.collective_compute(
                    "AllGather",
                    mybir.AluOpType.bypass,
                    replica_groups=[list(range(N_CORES))],
                    ins=[wu_in.opt()],
                    outs=[wu_out.opt()],
                )

            # resident lt: LAST RES_T vc-tiles.  Loaded on the scalar HWDGE
            # ring DURING step 1 (in parallel with the sync-ring stream of
            # the other groups), then reused DMA-free by steps 2-4.  One
            # tile per group so dependency tracking stays per-group.
            rs_g = []
            for i, g in enumerate(range(NG - res_g, NG)):
                t = persist.tile([P, NB * VLOC], mm_dt, name=f"rs{i}")
                nc.scalar.dma_start(
                    t[:].rearrange("p (u c) -> p u c", u=NB), lt_r[g])
                rs_g.append(t)

            def size_matched_warmup(src, tag):
                # re-syncs the ranks and warms the exact transfer shape of
                # the real per-step all-gathers
                wsb = work.tile([P, NB_S * C_IN], mm_dt,
                                name=f"wu_{tag}", tag="scs")
                nc.vector.tensor_copy(wsb[:], src[:, :NB_S * C_IN])
                win = dram.tile([VLOC, C_IN], mm_dt, name=f"wuin_{tag}")
                wout = dram.tile([V, C_IN], mm_dt, name=f"wuout_{tag}")
                nc.scalar.dma_start(
                    win.rearrange("(p j) c -> p j c", p=P),
                    wsb[:].rearrange("p (j c) -> p j c", j=NB_S))
                nc.gpsimd.collective_compute(
                    "AllGather",
                    mybir.AluOpType.bypass,
                    replica_groups=[list(range(N_CORES))],
                    ins=[win.opt()],
                    outs=[wout.opt()],
                )

            # group order: interleave streamed (S, DMA 4.7us / PE 2.6us)
            # and resident (R, PE-only 2.6us) so neither engine starves in
            # the PE-bound steps 2-4; the two surplus S's sit mid-step, and
            # the step ends on R's (DMA idles there -> prefetches the next
            # step through the boundary).
            n_s = NG - res_g
            s_list = list(range(n_s))
            r_list = list(range(n_s, NG))
            g_order = []
            si = ri = 0
            for p_i in range(res_g):
                g_order.append(s_list[si]); si += 1
                if p_i in (3, 7) and si < n_s:
                    g_order.append(s_list[si]); si += 1
                g_order.append(r_list[ri]); ri += 1
            g_order.extend(s_list[si:])
            # keep the final slot resident: swap any trailing S inward
            while g_order[-1] < n_s:
                g_order.insert(len(g_order) - 2, g_order.pop())

            for k in range(1, K_CHEB):
                acc = [accp.tile([C_IN, 512], f32, name=f"acc{k}_{ch}",
                                 tag="acc") for ch in range(N_CH)]
                for gi, g in enumerate(g_order):
                    if g >= NG - res_g:
                        src = rs_g[g - (NG - res_g)]
                        base = 0
                    else:
                        src = ltp.tile([P, NB * VLOC], mm_dt,
                                       name=f"lt{k}_{g}", tag="lt")
                        nc.sync.dma_start(
                            src[:].rearrange("p (u c) -> p u c", u=NB),
                            lt_r[g])
                        base = 0
                    for u in range(NB):
                        j = g * NB + u
                        st = sk_tiles[j // NB_S]
                        us = j % NB_S
                        for ch in range(N_CH):
                            nc.tensor.matmul(
                                acc[ch][:],
                                lhsT=st[:, us * C_IN:(us + 1) * C_IN],
                                rhs=src[:, base + u * VLOC + ch * 512:
                                        base + u * VLOC + (ch + 1) * 512],
                                start=(gi == 0 and u == 0),
                                stop=(gi == NG - 1 and u == NB - 1),
                            )
                    if k == 1 and g in (4, 9):
                        # size-matched warm-up AGs pinned inside step 1 so
                        # every real boundary AG runs at the warm floor
                        size_matched_warmup(src, f"w{g}")

                # ---- boundary, pipelined per 512-chunk:
                # T_k = 2*psum - T_{k-2} (step 1: copy), transpose, stage ----
                if k < K_CHEB - 1:
                    sc_stage = work.tile([P, (VLOC // P) * C_IN], mm_dt,
                                         name=f"scs{k}", tag="scs")
                tb = 32 * k
                for ch in range(N_CH):
                    sl = slice(ch * 512, (ch + 1) * 512)
                    if k == 1:
                        nc.vector.tensor_copy(t_ap(k)[:, sl], acc[ch][:])
                    else:
                        nc.vector.scalar_tensor_tensor(
                            t_ap(k)[:, sl], acc[ch][:], 2.0,
                            t_ap(k - 2)[:, sl],
                            mybir.AluOpType.mult, mybir.AluOpType.subtract)
                    if k < K_CHEB - 1:
                        for j2 in range(ch * 4, ch * 4 + 4):
                            tp_ps = tpp.tile([P, C_IN], f32,
                                             name=f"tp{k}_{j2}", tag="tp")
                            nc.tensor.transpose(
                                tp_ps[:],
                                t_blk[tb:tb + C_IN, j2 * P:(j2 + 1) * P],
                                id_sb[tb:tb + C_IN, :],
                                tile_position=(tb, 0) if tb == 96 else None)
                            nc.vector.tensor_copy(
                                sc_stage[:, j2 * C_IN:(j2 + 1) * C_IN],
                                tp_ps[:])

                if k < K_CHEB - 1:
                    cc_in = dram.tile([VLOC, C_IN], mm_dt, name=f"ccin{k}")
                    cc_out = dram.tile([V, C_IN], mm_dt, name=f"ccout{k}")
                    nc.scalar.dma_start(
                        cc_in.rearrange("(p j) c -> p j c", p=P),
                        sc_stage[:].rearrange("p (j c) -> p j c",
                                              j=VLOC // P))
                    nc.gpsimd.collective_compute(
                        "AllGather",
                        mybir.AluOpType.bypass,
                        replica_groups=[list(range(N_CORES))],
                        ins=[cc_in.opt()],
                        outs=[cc_out.opt()],
                    )
                    cc_r = cc_out.rearrange("(g p j) c -> g p j c",
                                            p=P, j=NB_S)
                    sk_tiles = load_stat(cc_r, k)

            # ---- out[o, v] = sum_k w_k^T @ T_k + bias ----
            # One matmul per chunk contracts ALL of T_0..T_3 (t_blk rows
            # 16-31 of each 32-block are zero, as are wf's), then T_4
            # accumulates on top; single bias add; store.
            for ch in range(N_CH):
                sl = slice(ch * 512, (ch + 1) * 512)
                ein = accp.tile([C_OUT, 512], f32, name=f"ein{ch}",
                                tag="acc")
                nc.tensor.matmul(ein[:], lhsT=w_sb[:], rhs=t_blk[:, sl],
                                 start=True, stop=False)
                nc.tensor.matmul(ein[:], lhsT=w4_sb[:], rhs=t4_sb[:, sl],
                                 start=False, stop=True)
                res = work.tile([C_OUT, 512], f32, name=f"res{ch}",
                                tag="res")
                nc.vector.tensor_scalar_add(res[:], ein[:], bias_sb[:])
                nc.scalar.dma_start(out.ap()[:, sl], res[:])

    nc.compile()
    return nc


def _interleave_rows(a, nb):
    """Within each nb*128-row group, reorder rows so row g*G+nb*p+u holds
    original row g*G+u*128+p (one contiguous per-partition read)."""
    ng = a.shape[0] // (P * nb)
    return np.ascontiguousarray(
        a.reshape(ng, nb, P, a.shape[1]).transpose(0, 2, 1, 3)
        .reshape(a.shape))


def _prep_inputs(x, L, weights, bias, cfg: str):
    np_dt = ml_dtypes.bfloat16 if cfg == "bf16" else np.float32
    x = np.asarray(x, dtype=np.float32)
    L = np.asarray(L, dtype=np.float32)
    weights = np.asarray(weights, dtype=np.float32)
    bias = np.asarray(bias, dtype=np.float32)

    Lt = np.ascontiguousarray(L.T).astype(np_dt)          # (V, V)
    xt = _interleave_rows(
        np.ascontiguousarray(x.T).astype(np_dt), NB_S)    # (V, C_IN)

    wf = np.zeros((P, C_OUT), dtype=np.float32)
    for k in range(K_CHEB - 1):
        wf[32 * k:32 * k + C_IN, :] = weights[k]
    w4 = np.ascontiguousarray(weights[K_CHEB - 1])
    b_ = np.ascontiguousarray(bias.reshape(C_OUT, 1))
    id128 = np.zeros((P, C_IN), dtype=np.float32)
    for p in range(P):
        if p % 32 < C_IN:
            id128[p, p % 32] = 1.0

    in_maps = []
    for d in range(N_CORES):
        cols = slice(d * VLOC, (d + 1) * VLOC)
        in_maps.append({
            "lt": _interleave_rows(np.ascontiguousarray(Lt[:, cols]), NB),
            "xt": xt,
            "xc": np.ascontiguousarray(x[:, cols]),
            "wf": wf,
            "w4": w4,
            "bias_in": b_,
            "id128": id128,
        })
    return in_maps


def run(x, L, weights, bias, cfg: str = "bf16", trace: bool = False,
        trace_cores=None):
    if cfg not in _CACHE:
        _CACHE[cfg] = _build(cfg)
    nc = _CACHE[cfg]
    in_maps = _prep_inputs(x, L, weights, bias, cfg)
    kw = {}
    if trace_cores is not None:
        kw["trace_cores"] = trace_cores
    res = bass_utils.run_bass_kernel_spmd(
        nc, in_maps, core_ids=list(range(N_CORES)), trace=trace, **kw)
    out = np.concatenate([res.results[d]["out"] for d in range(N_CORES)],
                         axis=1)
    return out.astype(np.float32), res


def kernel(x, L, weights, bias):
    out, _ = run(x, L, weights, bias, cfg="bf16")
    return out


# revision 7
# speedup vs baseline: 1.1187x; 1.1187x over previous
import sys

if "/opt/trn_rl_repo" not in sys.path:
    sys.path.insert(0, "/opt/trn_rl_repo")

import numpy as np
import ml_dtypes

import concourse.bass as bass
import concourse.bacc as bacc
import concourse.tile as tile
import concourse.mybir as mybir
from concourse import bass_utils

# Problem shapes (nn_ChebConv): x (16, 12288), L (12288, 12288),
# weights (5, 16, 32), bias (32,). out (32, 12288).
#
# Sharding: core d owns V-columns [d*1536, (d+1)*1536).  Host feeds each
# core lt = L^T[:, cols_d] (contraction dim on partitions), row-interleaved
# within 512-row groups so each SBUF partition reads one contiguous chunk
# per bulk DMA.
#
# Schedule: step 1 streams ALL 96 vc-tiles (37.7 MB) at full HBM rate on
# two parallel queues (streamed tiles -> ltp pool on sync, resident tiles
# -> rs_sb on vector), with the PE chasing the stream; steps 2-4 touch
# only the 52 streamed tiles (resident 44 stay in SBUF), making them
# PE-bound (~62us each).  Step boundary: per-chunk DVE + PE-transpose +
# stage, one small AllGather (warmed to ~7us by 4 warm-up AGs), stationary
# reload.  Tail: one fused einsum matmul per 512-chunk contracting all of
# T_0..T_3 at once (w stacked on 128 partitions) + T_4 accumulate + bias.
C_IN = 16
C_OUT = 32
K_CHEB = 5
V = 12288
N_CORES = 8
VLOC = V // N_CORES          # 1536 columns of the V axis per core
P = 128
NT_VC = V // P               # 96 contraction tiles per step
N_CH = VLOC // 512           # 3 psum chunks of 512
NB = 4                       # vc-tiles per bulk lt DMA (512 rows)
NG = NT_VC // NB             # 24 groups
RES_T = 44                   # vc-tiles resident in SBUF (of 96)
RES_G = RES_T // NB          # 11 resident groups
LT_BUFS = 3
NB_S = VLOC // P             # stationary group: 12 vc-tiles = one rank chunk
NG_S = V // (P * NB_S)       # 8 stationary groups

_CACHE: dict = {}


def _build(cfg: str):
    if cfg == "bf16":
        mm_dt = mybir.dt.bfloat16
    else:
        mm_dt = mybir.dt.float32
    f32 = mybir.dt.float32

    res_g = RES_G

    nc = bacc.Bacc("TRN2", target_bir_lowering=False, debug=False,
                   num_devices=N_CORES)

    lt = nc.dram_tensor("lt", [V, VLOC], mm_dt, kind="ExternalInput")
    xt = nc.dram_tensor("xt", [V, C_IN], mm_dt, kind="ExternalInput")
    xc = nc.dram_tensor("xc", [C_IN, VLOC], f32, kind="ExternalInput")
    wf = nc.dram_tensor("wf", [P, C_OUT], f32, kind="ExternalInput")
    w4 = nc.dram_tensor("w4", [C_IN, C_OUT], f32, kind="ExternalInput")
    bias_in = nc.dram_tensor("bias_in", [C_OUT, 1], f32, kind="ExternalInput")
    id128 = nc.dram_tensor("id128", [P, C_IN], f32, kind="ExternalInput")
    out = nc.dram_tensor("out", [C_OUT, VLOC], f32, kind="ExternalOutput")

    lt_r = lt.ap().rearrange("(g p u) c -> g p u c", p=P, u=NB)

    with tile.TileContext(nc) as tc:
        with (
            tc.tile_pool(name="ltp", bufs=LT_BUFS) as ltp,
            tc.tile_pool(name="persist", bufs=1) as persist,
            tc.tile_pool(name="stat", bufs=2 * NG_S) as statp,
            tc.tile_pool(name="work", bufs=2) as work,
            tc.tile_pool(name="acc", bufs=4, space="PSUM") as accp,
            tc.tile_pool(name="tpp", bufs=4, space="PSUM") as tpp,
            tc.tile_pool(name="dram", bufs=1, space="DRAM") as dram,
        ):
            # ---- persistent small tensors ----
            w_sb = persist.tile([P, C_OUT], f32)
            nc.scalar.dma_start(w_sb[:], wf.ap())
            w4_sb = persist.tile([C_IN, C_OUT], f32)
            nc.scalar.dma_start(w4_sb[:], w4.ap())
            bias_sb = persist.tile([C_OUT, 1], f32)
            nc.scalar.dma_start(bias_sb[:], bias_in.ap())
            id_sb = persist.tile([P, C_IN], f32)
            nc.scalar.dma_start(id_sb[:], id128.ap())

            # T_0..T_3 stacked at partition bases {0,32,64,96} of one tile
            # (rows 16-31 of each block must be ZERO: the fused einsum
            # contracts all 128 partitions against zero-padded weights).
            t_blk = persist.tile([P, VLOC], f32)
            t4_sb = persist.tile([C_IN, VLOC], f32)
            nc.vector.memset(t_blk[:], 0.0)
            nc.scalar.dma_start(t_blk[0:C_IN, :], xc.ap())

            def t_ap(k):
                if k == K_CHEB - 1:
                    return t4_sb[:]
                return t_blk[32 * k:32 * k + C_IN, :]

            # stationary tensors (x^T, then each gathered T_k^T) live in
            # rank-aligned 1536-row groups, row-interleaved so partition p
            # reads rows [g*1536 + 12p, +12) — one 384 B chunk.
            def load_stat(src_r, tag_k):
                tiles = []
                for g in range(NG_S):
                    s = statp.tile([P, NB_S * C_IN], mm_dt,
                                   name=f"st{tag_k}_{g}", tag="stat")
                    nc.scalar.dma_start(
                        s[:].rearrange("p (j c) -> p j c", j=NB_S), src_r[g])
                    tiles.append(s)
                return tiles

            xt_r = xt.ap().rearrange("(g p j) c -> g p j c", p=P, j=NB_S)
            sk_tiles = load_stat(xt_r, 0)

            # tiny warm-up AllGathers: pay the ~70us first-collective cost
            # while step 1 streams, not on the critical path
            wu_sb = work.tile([P, C_IN], mm_dt, name="wu_sb", tag="wu")
            nc.vector.memset(wu_sb[:], 0.0)
            for w in range(2):
                wu_in = dram.tile([P, C_IN], mm_dt, name=f"wu_in{w}")
                wu_out = dram.tile([P * N_CORES, C_IN], mm_dt,
                                   name=f"wu_out{w}")
                nc.scalar.dma_start(wu_in[:], wu_sb[:])
                nc.gpsimd.collective_compute(
                    "AllGather",
                    mybir.AluOpType.bypass,
                    replica_groups=[list(range(N_CORES))],
                    ins=[wu_in.opt()],
                    outs=[wu_out.opt()],
                )

            # resident lt: LAST RES_T vc-tiles.  Loaded just-in-time during
            # step 1 on the same sync HWDGE ring as the stream (issue order
            # = consumption order; HBM is the shared bottleneck), then
            # reused DMA-free by steps 2-4.  The scalar ring stays light so
            # the one-time CC setup isn't starved.
            rs_g = [persist.tile([P, NB * VLOC], mm_dt, name=f"rs{i}")
                    for i in range(res_g)]

            def size_matched_warmup(src, tag):
                # re-syncs the ranks and warms the exact transfer shape of
                # the real per-step all-gathers
                wsb = work.tile([P, NB_S * C_IN], mm_dt,
                                name=f"wu_{tag}", tag="scs")
                nc.vector.tensor_copy(wsb[:], src[:, :NB_S * C_IN])
                win = dram.tile([VLOC, C_IN], mm_dt, name=f"wuin_{tag}")
                wout = dram.tile([V, C_IN], mm_dt, name=f"wuout_{tag}")
                nc.scalar.dma_start(
                    win.rearrange("(p j) c -> p j c", p=P),
                    wsb[:].rearrange("p (j c) -> p j c", j=NB_S))
                nc.gpsimd.collective_compute(
                    "AllGather",
                    mybir.AluOpType.bypass,
                    replica_groups=[list(range(N_CORES))],
                    ins=[win.opt()],
                    outs=[wout.opt()],
                )

            # group order: interleave streamed (S, DMA 4.7us / PE 2.6us)
            # and resident (R, PE-only 2.6us) so neither engine starves in
            # the PE-bound steps 2-4; the two surplus S's sit mid-step, and
            # the step ends on R's (DMA idles there -> prefetches the next
            # step through the boundary).
            n_s = NG - res_g
            s_list = list(range(n_s))
            r_list = list(range(n_s, NG))
            g_order = []
            si = ri = 0
            for p_i in range(res_g):
                g_order.append(s_list[si]); si += 1
                if p_i in (3, 7) and si < n_s:
                    g_order.append(s_list[si]); si += 1
                g_order.append(r_list[ri]); ri += 1
            g_order.extend(s_list[si:])
            # keep the final slot resident: swap any trailing S inward
            while g_order[-1] < n_s:
                g_order.insert(len(g_order) - 2, g_order.pop())

            for k in range(1, K_CHEB):
                acc = [accp.tile([C_IN, 512], f32, name=f"acc{k}_{ch}",
                                 tag="acc") for ch in range(N_CH)]
                for gi, g in enumerate(g_order):
                    if g >= NG - res_g:
                        src = rs_g[g - (NG - res_g)]
                        if k == 1:
                            nc.sync.dma_start(
                                src[:].rearrange("p (u c) -> p u c", u=NB),
                                lt_r[g])
                        base = 0
                    else:
                        src = ltp.tile([P, NB * VLOC], mm_dt,
                                       name=f"lt{k}_{g}", tag="lt")
                        nc.sync.dma_start(
                            src[:].rearrange("p (u c) -> p u c", u=NB),
                            lt_r[g])
                        base = 0
                    for u in range(NB):
                        j = g * NB + u
                        st = sk_tiles[j // NB_S]
                        us = j % NB_S
                        for ch in range(N_CH):
                            nc.tensor.matmul(
                                acc[ch][:],
                                lhsT=st[:, us * C_IN:(us + 1) * C_IN],
                                rhs=src[:, base + u * VLOC + ch * 512:
                                        base + u * VLOC + (ch + 1) * 512],
                                start=(gi == 0 and u == 0),
                                stop=(gi == NG - 1 and u == NB - 1),
                            )
                    if k == 1 and g in (4, 9):
                        # size-matched warm-up AGs pinned inside step 1 so
                        # every real boundary AG runs at the warm floor
                        size_matched_warmup(src, f"w{g}")

                # ---- boundary, pipelined per 512-chunk:
                # T_k = 2*psum - T_{k-2} (step 1: copy), transpose, stage ----
                if k < K_CHEB - 1:
                    sc_stage = work.tile([P, (VLOC // P) * C_IN], mm_dt,
                                         name=f"scs{k}", tag="scs")
                tb = 32 * k
                for ch in range(N_CH):
                    sl = slice(ch * 512, (ch + 1) * 512)
                    if k == 1:
                        nc.vector.tensor_copy(t_ap(k)[:, sl], acc[ch][:])
                    else:
                        nc.vector.scalar_tensor_tensor(
                            t_ap(k)[:, sl], acc[ch][:], 2.0,
                            t_ap(k - 2)[:, sl],
                            mybir.AluOpType.mult, mybir.AluOpType.subtract)
                    if k < K_CHEB - 1:
                        for j2 in range(ch * 4, ch * 4 + 4):
                            tp_ps = tpp.tile([P, C_IN], f32,
                                             name=f"tp{k}_{j2}", tag="tp")
                            nc.tensor.transpose(
                                tp_ps[:],
                                t_blk[tb:tb + C_IN, j2 * P:(j2 + 1) * P],
                                id_sb[tb:tb + C_IN, :],
                                tile_position=(tb, 0) if tb == 96 else None)
                            nc.vector.tensor_copy(
                                sc_stage[:, j2 * C_IN:(j2 + 1) * C_IN],
                                tp_ps[:])

                if k < K_CHEB - 1:
                    cc_in = dram.tile([VLOC, C_IN], mm_dt, name=f"ccin{k}")
                    cc_out = dram.tile([V, C_IN], mm_dt, name=f"ccout{k}")
                    nc.scalar.dma_start(
                        cc_in.rearrange("(p j) c -> p j c", p=P),
                        sc_stage[:].rearrange("p (j c) -> p j c",
                                              j=VLOC // P))
                    nc.gpsimd.collective_compute(
                        "AllGather",
                        mybir.AluOpType.bypass,
                        replica_groups=[list(range(N_CORES))],
                        ins=[cc_in.opt()],
                        outs=[cc_out.opt()],
                    )
                    cc_r = cc_out.rearrange("(g p j) c -> g p j c",
                                            p=P, j=NB_S)
                    sk_tiles = load_stat(cc_r, k)

            # ---- out[o, v] = sum_k w_k^T @ T_k + bias ----
            # One matmul per chunk contracts ALL of T_0..T_3 (t_blk rows
            # 16-31 of each 32-block are zero, as are wf's), then T_4
            # accumulates on top; single bias add; store.
            for ch in range(N_CH):
                sl = slice(ch * 512, (ch + 1) * 512)
                ein = accp.tile([C_OUT, 512], f32, name=f"ein{ch}",
                                tag="acc")
                nc.tensor.matmul(ein[:], lhsT=w_sb[:], rhs=t_blk[:, sl],
                                 start=True, stop=False)
                nc.tensor.matmul(ein[:], lhsT=w4_sb[:], rhs=t4_sb[:, sl],
                                 start=False, stop=True)
                res = work.tile([C_OUT, 512], f32, name=f"res{ch}",
                                tag="res")
                nc.vector.tensor_scalar_add(res[:], ein[:], bias_sb[:])
                nc.scalar.dma_start(out.ap()[:, sl], res[:])

    nc.compile()
    return nc


def _interleave_rows(a, nb):
    """Within each nb*128-row group, reorder rows so row g*G+nb*p+u holds
    original row g*G+u*128+p (one contiguous per-partition read)."""
    ng = a.shape[0] // (P * nb)
    return np.ascontiguousarray(
        a.reshape(ng, nb, P, a.shape[1]).transpose(0, 2, 1, 3)
        .reshape(a.shape))


def _prep_inputs(x, L, weights, bias, cfg: str):
    np_dt = ml_dtypes.bfloat16 if cfg == "bf16" else np.float32
    x = np.asarray(x, dtype=np.float32)
    L = np.asarray(L, dtype=np.float32)
    weights = np.asarray(weights, dtype=np.float32)
    bias = np.asarray(bias, dtype=np.float32)

    Lt = np.ascontiguousarray(L.T).astype(np_dt)          # (V, V)
    xt = _interleave_rows(
        np.ascontiguousarray(x.T).astype(np_dt), NB_S)    # (V, C_IN)

    wf = np.zeros((P, C_OUT), dtype=np.float32)
    for k in range(K_CHEB - 1):
        wf[32 * k:32 * k + C_IN, :] = weights[k]
    w4 = np.ascontiguousarray(weights[K_CHEB - 1])
    b_ = np.ascontiguousarray(bias.reshape(C_OUT, 1))
    id128 = np.zeros((P, C_IN), dtype=np.float32)
    for p in range(P):
        if p % 32 < C_IN:
            id128[p, p % 32] = 1.0

    in_maps = []
    for d in range(N_CORES):
        cols = slice(d * VLOC, (d + 1) * VLOC)
        in_maps.append({
            "lt": _interleave_rows(np.ascontiguousarray(Lt[:, cols]), NB),
            "xt": xt,
            "xc": np.ascontiguousarray(x[:, cols]),
            "wf": wf,
            "w4": w4,
            "bias_in": b_,
            "id128": id128,
        })
    return in_maps


def run(x, L, weights, bias, cfg: str = "bf16", trace: bool = False,
        trace_cores=None):
    if cfg not in _CACHE:
        _CACHE[cfg] = _build(cfg)
    nc = _CACHE[cfg]
    in_maps = _prep_inputs(x, L, weights, bias, cfg)
    kw = {}
    if trace_cores is not None:
        kw["trace_cores"] = trace_cores
    res = bass_utils.run_bass_kernel_spmd(
        nc, in_maps, core_ids=list(range(N_CORES)), trace=trace, **kw)
    out = np.concatenate([res.results[d]["out"] for d in range(N_CORES)],
                         axis=1)
    return out.astype(np.float32), res


def kernel(x, L, weights, bias):
    out, _ = run(x, L, weights, bias, cfg="bf16")
    return out


# revision 12
# speedup vs baseline: 1.1482x; 1.0263x over previous
import sys

if "/opt/trn_rl_repo" not in sys.path:
    sys.path.insert(0, "/opt/trn_rl_repo")

import numpy as np
import ml_dtypes

import concourse.bass as bass
import concourse.bacc as bacc
import concourse.tile as tile
import concourse.mybir as mybir
from concourse import bass_utils

# Problem shapes (nn_ChebConv): x (16, 12288), L (12288, 12288),
# weights (5, 16, 32), bias (32,). out (32, 12288).
#
# Sharding: core d owns V-columns [d*1536, (d+1)*1536).  Host feeds each
# core lt = L^T[:, cols_d] (contraction dim on partitions), row-interleaved
# within 512-row groups so each SBUF partition reads one contiguous chunk
# per bulk DMA.
#
# Schedule: step 1 streams ALL 96 vc-tiles (37.7 MB) at full HBM rate on
# two parallel queues (streamed tiles -> ltp pool on sync, resident tiles
# -> rs_sb on vector), with the PE chasing the stream; steps 2-4 touch
# only the 52 streamed tiles (resident 44 stay in SBUF), making them
# PE-bound (~62us each).  Step boundary: per-chunk DVE + PE-transpose +
# stage, one small AllGather (warmed to ~7us by 4 warm-up AGs), stationary
# reload.  Tail: one fused einsum matmul per 512-chunk contracting all of
# T_0..T_3 at once (w stacked on 128 partitions) + T_4 accumulate + bias.
C_IN = 16
C_OUT = 32
K_CHEB = 5
V = 12288
N_CORES = 8
VLOC = V // N_CORES          # 1536 columns of the V axis per core
P = 128
NT_VC = V // P               # 96 contraction tiles per step
N_CH = VLOC // 512           # 3 psum chunks of 512
NB = 4                       # vc-tiles per bulk lt DMA (512 rows)
NG = NT_VC // NB             # 24 groups
RES_T = 44                   # vc-tiles resident in SBUF (of 96)
RES_G = RES_T // NB          # 11 resident groups
LT_BUFS = 4
NB_S = VLOC // P             # stationary group: 12 vc-tiles = one rank chunk
NG_S = V // (P * NB_S)       # 8 stationary groups

_CACHE: dict = {}


def _build(cfg: str):
    if cfg == "bf16":
        mm_dt = mybir.dt.bfloat16
    else:
        mm_dt = mybir.dt.float32
    f32 = mybir.dt.float32

    res_g = RES_G

    nc = bacc.Bacc("TRN2", target_bir_lowering=False, debug=False,
                   num_devices=N_CORES)

    lt = nc.dram_tensor("lt", [V, VLOC], mm_dt, kind="ExternalInput")
    xt = nc.dram_tensor("xt", [V, C_IN], mm_dt, kind="ExternalInput")
    xc = nc.dram_tensor("xc", [C_IN, VLOC], f32, kind="ExternalInput")
    wf = nc.dram_tensor("wf", [P, C_OUT], f32, kind="ExternalInput")
    w4 = nc.dram_tensor("w4", [C_IN, C_OUT], f32, kind="ExternalInput")
    bias_in = nc.dram_tensor("bias_in", [C_OUT, 1], f32, kind="ExternalInput")
    id128 = nc.dram_tensor("id128", [P, C_IN], f32, kind="ExternalInput")
    out = nc.dram_tensor("out", [C_OUT, VLOC], f32, kind="ExternalOutput")

    lt_r = lt.ap().rearrange("(g p u) c -> g p u c", p=P, u=NB)

    with tile.TileContext(nc) as tc:
        with (
            tc.tile_pool(name="ltp", bufs=LT_BUFS) as ltp,
            tc.tile_pool(name="persist", bufs=1) as persist,
            tc.tile_pool(name="stat", bufs=2 * NG_S) as statp,
            tc.tile_pool(name="work", bufs=2) as work,
            tc.tile_pool(name="acc", bufs=4, space="PSUM") as accp,
            tc.tile_pool(name="tpp", bufs=4, space="PSUM") as tpp,
            tc.tile_pool(name="dram", bufs=1, space="DRAM") as dram,
        ):
            # ---- persistent small tensors ----
            w_sb = persist.tile([P, C_OUT], f32)
            nc.scalar.dma_start(w_sb[:], wf.ap())
            w4_sb = persist.tile([C_IN, C_OUT], f32)
            nc.scalar.dma_start(w4_sb[:], w4.ap())
            bias_sb = persist.tile([C_OUT, 1], f32)
            nc.scalar.dma_start(bias_sb[:], bias_in.ap())
            id_sb = persist.tile([P, C_IN], f32)
            nc.scalar.dma_start(id_sb[:], id128.ap())

            # T_0..T_3 stacked at partition bases {0,32,64,96} of one tile
            # (rows 16-31 of each block must be ZERO: the fused einsum
            # contracts all 128 partitions against zero-padded weights).
            t_blk = persist.tile([P, VLOC], f32)
            t4_sb = persist.tile([C_IN, VLOC], f32)
            nc.vector.memset(t_blk[:], 0.0)
            nc.scalar.dma_start(t_blk[0:C_IN, :], xc.ap())

            def t_ap(k):
                if k == K_CHEB - 1:
                    return t4_sb[:]
                return t_blk[32 * k:32 * k + C_IN, :]

            # stationary tensors (x^T, then each gathered T_k^T) live in
            # rank-aligned 1536-row groups, row-interleaved so partition p
            # reads rows [g*1536 + 12p, +12) — one 384 B chunk.
            def load_stat(src_r, tag_k):
                tiles = []
                for g in range(NG_S):
                    s = statp.tile([P, NB_S * C_IN], mm_dt,
                                   name=f"st{tag_k}_{g}", tag="stat")
                    nc.scalar.dma_start(
                        s[:].rearrange("p (j c) -> p j c", j=NB_S), src_r[g])
                    tiles.append(s)
                return tiles

            xt_r = xt.ap().rearrange("(g p j) c -> g p j c", p=P, j=NB_S)
            sk_tiles = load_stat(xt_r, 0)

            # tiny warm-up AllGathers: pay the ~70us first-collective cost
            # while step 1 streams, not on the critical path
            wu_sb = work.tile([P, C_IN], mm_dt, name="wu_sb", tag="wu")
            nc.vector.memset(wu_sb[:], 0.0)
            wu_in = dram.tile([P, C_IN], mm_dt, name="wu_in0")
            wu_out = dram.tile([P * N_CORES, C_IN], mm_dt, name="wu_out0")
            nc.scalar.dma_start(wu_in[:], wu_sb[:])
            nc.gpsimd.collective_compute(
                "AllGather",
                mybir.AluOpType.bypass,
                replica_groups=[list(range(N_CORES))],
                ins=[wu_in.opt()],
                outs=[wu_out.opt()],
            )

            # resident lt: LAST RES_T vc-tiles.  Loaded just-in-time during
            # step 1 on the same sync HWDGE ring as the stream (issue order
            # = consumption order; HBM is the shared bottleneck), then
            # reused DMA-free by steps 2-4.  The scalar ring stays light so
            # the one-time CC setup isn't starved.
            rs_g = [persist.tile([P, NB * VLOC], mm_dt, name=f"rs{i}")
                    for i in range(res_g)]

            def size_matched_warmup(src, tag):
                # re-syncs the ranks and warms the exact transfer shape of
                # the real per-step all-gathers
                wsb = work.tile([P, NB_S * C_IN], mm_dt,
                                name=f"wu_{tag}", tag="scs")
                nc.vector.tensor_copy(wsb[:], src[:, :NB_S * C_IN])
                win = dram.tile([VLOC, C_IN], mm_dt, name=f"wuin_{tag}")
                wout = dram.tile([V, C_IN], mm_dt, name=f"wuout_{tag}")
                nc.scalar.dma_start(
                    win.rearrange("(p j) c -> p j c", p=P),
                    wsb[:].rearrange("p (j c) -> p j c", j=NB_S))
                nc.gpsimd.collective_compute(
                    "AllGather",
                    mybir.AluOpType.bypass,
                    replica_groups=[list(range(N_CORES))],
                    ins=[win.opt()],
                    outs=[wout.opt()],
                )

            # group order: interleave streamed (S, DMA 4.7us / PE 2.6us)
            # and resident (R, PE-only 2.6us) so neither engine starves in
            # the PE-bound steps 2-4; the two surplus S's sit mid-step, and
            # the step ends on R's (DMA idles there -> prefetches the next
            # step through the boundary).
            n_s = NG - res_g
            s_list = list(range(n_s))
            r_list = list(range(n_s, NG))
            g_order = []
            si = ri = 0
            for p_i in range(res_g):
                g_order.append(s_list[si]); si += 1
                if p_i in (3, 7) and si < n_s:
                    g_order.append(s_list[si]); si += 1
                g_order.append(r_list[ri]); ri += 1
            g_order.extend(s_list[si:])
            # keep the final slot resident: swap any trailing S inward
            while g_order[-1] < n_s:
                g_order.insert(len(g_order) - 2, g_order.pop())

            ein = None
            for k in range(1, K_CHEB):
                acc = [accp.tile([C_IN, 512], f32, name=f"acc{k}_{ch}",
                                 tag="acc") for ch in range(N_CH)]
                if k == K_CHEB - 1:
                    # T_0..T_3 einsum term: one matmul per chunk contracting
                    # all 128 t_blk partitions (zero-padded rows match wf's
                    # zeros).  Runs during step 4 on the idle transpose-pool
                    # banks; the T_4 term accumulates on top in the tail.
                    ein = [tpp.tile([C_OUT, 512], f32, name=f"ein{ch}",
                                    tag="tp") for ch in range(N_CH)]
                    for ch in range(N_CH):
                        nc.tensor.matmul(
                            ein[ch][:], lhsT=w_sb[:],
                            rhs=t_blk[:, ch * 512:(ch + 1) * 512],
                            start=True, stop=False)
                for gi, g in enumerate(g_order):
                    if g >= NG - res_g:
                        src = rs_g[g - (NG - res_g)]
                        if k == 1:
                            nc.sync.dma_start(
                                src[:].rearrange("p (u c) -> p u c", u=NB),
                                lt_r[g])
                        base = 0
                    else:
                        src = ltp.tile([P, NB * VLOC], mm_dt,
                                       name=f"lt{k}_{g}", tag="lt")
                        nc.sync.dma_start(
                            src[:].rearrange("p (u c) -> p u c", u=NB),
                            lt_r[g])
                        base = 0
                    # final group runs chunk-major so psum chunks drain
                    # staggered and the boundary DVE/transposes overlap the
                    # remaining matmuls
                    if gi == NG - 1:
                        uch = [(u, ch) for ch in range(N_CH)
                               for u in range(NB)]
                    else:
                        uch = [(u, ch) for u in range(NB)
                               for ch in range(N_CH)]
                    for u, ch in uch:
                        j = g * NB + u
                        st = sk_tiles[j // NB_S]
                        us = j % NB_S
                        nc.tensor.matmul(
                            acc[ch][:],
                            lhsT=st[:, us * C_IN:(us + 1) * C_IN],
                            rhs=src[:, base + u * VLOC + ch * 512:
                                    base + u * VLOC + (ch + 1) * 512],
                            start=(gi == 0 and u == 0),
                            stop=(gi == NG - 1 and u == NB - 1),
                        )
                    if k == 1 and g == 4:
                        # one size-matched warm-up AG pinned inside step 1
                        # so the real boundary AGs run at the warm floor
                        size_matched_warmup(src, f"w{g}")

                # ---- boundary, pipelined per 512-chunk:
                # T_k = 2*psum - T_{k-2} (step 1: copy), transpose, stage ----
                if k < K_CHEB - 1:
                    sc_stage = work.tile([P, (VLOC // P) * C_IN], mm_dt,
                                         name=f"scs{k}", tag="scs")
                tb = 32 * k
                for ch in range(N_CH):
                    sl = slice(ch * 512, (ch + 1) * 512)
                    if k == 1:
                        nc.vector.tensor_copy(t_ap(k)[:, sl], acc[ch][:])
                    else:
                        nc.vector.scalar_tensor_tensor(
                            t_ap(k)[:, sl], acc[ch][:], 2.0,
                            t_ap(k - 2)[:, sl],
                            mybir.AluOpType.mult, mybir.AluOpType.subtract)
                    if k < K_CHEB - 1:
                        for j2 in range(ch * 4, ch * 4 + 4):
                            tp_ps = tpp.tile([P, C_IN], f32,
                                             name=f"tp{k}_{j2}", tag="tp")
                            nc.tensor.transpose(
                                tp_ps[:],
                                t_blk[tb:tb + C_IN, j2 * P:(j2 + 1) * P],
                                id_sb[tb:tb + C_IN, :],
                                tile_position=(tb, 0) if tb == 96 else None)
                            nc.vector.tensor_copy(
                                sc_stage[:, j2 * C_IN:(j2 + 1) * C_IN],
                                tp_ps[:])

                if k < K_CHEB - 1:
                    cc_in = dram.tile([VLOC, C_IN], mm_dt, name=f"ccin{k}")
                    cc_out = dram.tile([V, C_IN], mm_dt, name=f"ccout{k}")
                    nc.scalar.dma_start(
                        cc_in.rearrange("(p j) c -> p j c", p=P),
                        sc_stage[:].rearrange("p (j c) -> p j c",
                                              j=VLOC // P))
                    nc.gpsimd.collective_compute(
                        "AllGather",
                        mybir.AluOpType.bypass,
                        replica_groups=[list(range(N_CORES))],
                        ins=[cc_in.opt()],
                        outs=[cc_out.opt()],
                    )
                    cc_r = cc_out.rearrange("(g p j) c -> g p j c",
                                            p=P, j=NB_S)
                    sk_tiles = load_stat(cc_r, k)

            # ---- tail: accumulate the T_4 term per chunk, add bias, store
            for ch in range(N_CH):
                sl = slice(ch * 512, (ch + 1) * 512)
                nc.tensor.matmul(ein[ch][:], lhsT=w4_sb[:], rhs=t4_sb[:, sl],
                                 start=False, stop=True)
                res = work.tile([C_OUT, 512], f32, name=f"res{ch}",
                                tag="res")
                nc.vector.tensor_scalar_add(res[:], ein[ch][:], bias_sb[:])
                nc.scalar.dma_start(out.ap()[:, sl], res[:])

    nc.compile()
    return nc


def _interleave_rows(a, nb):
    """Within each nb*128-row group, reorder rows so row g*G+nb*p+u holds
    original row g*G+u*128+p (one contiguous per-partition read)."""
    ng = a.shape[0] // (P * nb)
    return np.ascontiguousarray(
        a.reshape(ng, nb, P, a.shape[1]).transpose(0, 2, 1, 3)
        .reshape(a.shape))


def _prep_inputs(x, L, weights, bias, cfg: str):
    np_dt = ml_dtypes.bfloat16 if cfg == "bf16" else np.float32
    x = np.asarray(x, dtype=np.float32)
    L = np.asarray(L, dtype=np.float32)
    weights = np.asarray(weights, dtype=np.float32)
    bias = np.asarray(bias, dtype=np.float32)

    Lt = np.ascontiguousarray(L.T).astype(np_dt)          # (V, V)
    xt = _interleave_rows(
        np.ascontiguousarray(x.T).astype(np_dt), NB_S)    # (V, C_IN)

    wf = np.zeros((P, C_OUT), dtype=np.float32)
    for k in range(K_CHEB - 1):
        wf[32 * k:32 * k + C_IN, :] = weights[k]
    w4 = np.ascontiguousarray(weights[K_CHEB - 1])
    b_ = np.ascontiguousarray(bias.reshape(C_OUT, 1))
    id128 = np.zeros((P, C_IN), dtype=np.float32)
    for p in range(P):
        if p % 32 < C_IN:
            id128[p, p % 32] = 1.0

    in_maps = []
    for d in range(N_CORES):
        cols = slice(d * VLOC, (d + 1) * VLOC)
        in_maps.append({
            "lt": _interleave_rows(np.ascontiguousarray(Lt[:, cols]), NB),
            "xt": xt,
            "xc": np.ascontiguousarray(x[:, cols]),
            "wf": wf,
            "w4": w4,
            "bias_in": b_,
            "id128": id128,
        })
    return in_maps


def run(x, L, weights, bias, cfg: str = "bf16", trace: bool = False,
        trace_cores=None):
    if cfg not in _CACHE:
        _CACHE[cfg] = _build(cfg)
    nc = _CACHE[cfg]
    in_maps = _prep_inputs(x, L, weights, bias, cfg)
    kw = {}
    if trace_cores is not None:
        kw["trace_cores"] = trace_cores
    res = bass_utils.run_bass_kernel_spmd(
        nc, in_maps, core_ids=list(range(N_CORES)), trace=trace, **kw)
    out = np.concatenate([res.results[d]["out"] for d in range(N_CORES)],
                         axis=1)
    return out.astype(np.float32), res


def kernel(x, L, weights, bias):
    out, _ = run(x, L, weights, bias, cfg="bf16")
    return out


# revision 15
# speedup vs baseline: 1.1761x; 1.0244x over previous
import sys

if "/opt/trn_rl_repo" not in sys.path:
    sys.path.insert(0, "/opt/trn_rl_repo")

import numpy as np
import ml_dtypes

import concourse.bass as bass
import concourse.bacc as bacc
import concourse.tile as tile
import concourse.mybir as mybir
from concourse import bass_utils

# Problem shapes (nn_ChebConv): x (16, 12288), L (12288, 12288),
# weights (5, 16, 32), bias (32,). out (32, 12288).
#
# Sharding: core d owns V-columns [d*1536, (d+1)*1536).  Host feeds each
# core lt = L^T[:, cols_d] (contraction dim on partitions), row-interleaved
# within 512-row groups so each SBUF partition reads one contiguous chunk
# per bulk DMA.
#
# Schedule: step 1 streams ALL 96 vc-tiles (37.7 MB) at full HBM rate on
# two parallel queues (streamed tiles -> ltp pool on sync, resident tiles
# -> rs_sb on vector), with the PE chasing the stream; steps 2-4 touch
# only the 52 streamed tiles (resident 44 stay in SBUF), making them
# PE-bound (~62us each).  Step boundary: per-chunk DVE + PE-transpose +
# stage, one small AllGather (warmed to ~7us by 4 warm-up AGs), stationary
# reload.  Tail: one fused einsum matmul per 512-chunk contracting all of
# T_0..T_3 at once (w stacked on 128 partitions) + T_4 accumulate + bias.
C_IN = 16
C_OUT = 32
K_CHEB = 5
V = 12288
N_CORES = 8
VLOC = V // N_CORES          # 1536 columns of the V axis per core
P = 128
NT_VC = V // P               # 96 contraction tiles per step
N_CH = VLOC // 512           # 3 psum chunks of 512
NB = 4                       # vc-tiles per bulk lt DMA (512 rows)
NG = NT_VC // NB             # 24 groups
RES_T = 44                   # vc-tiles resident in SBUF (of 96)
RES_G = RES_T // NB          # 11 resident groups
LT_BUFS = 4
NB_S = VLOC // P             # stationary group: 12 vc-tiles = one rank chunk
NG_S = V // (P * NB_S)       # 8 stationary groups

_CACHE: dict = {}


def _build(cfg: str):
    if cfg == "bf16":
        mm_dt = mybir.dt.bfloat16
    else:
        mm_dt = mybir.dt.float32
    f32 = mybir.dt.float32

    res_g = RES_G

    nc = bacc.Bacc("TRN2", target_bir_lowering=False, debug=False,
                   num_devices=N_CORES)

    lt = nc.dram_tensor("lt", [V, VLOC], mm_dt, kind="ExternalInput")
    xt = nc.dram_tensor("xt", [V, C_IN], mm_dt, kind="ExternalInput")
    xc = nc.dram_tensor("xc", [C_IN, VLOC], mm_dt, kind="ExternalInput")
    wf = nc.dram_tensor("wf", [P, C_OUT], mm_dt, kind="ExternalInput")
    w4 = nc.dram_tensor("w4", [C_IN, C_OUT], mm_dt, kind="ExternalInput")
    bias_in = nc.dram_tensor("bias_in", [1, C_OUT], f32, kind="ExternalInput")
    id128 = nc.dram_tensor("id128", [P, C_IN], mm_dt, kind="ExternalInput")
    out = nc.dram_tensor("out", [C_OUT, VLOC], f32, kind="ExternalOutput")

    lt_r = lt.ap().rearrange("(g p u) c -> g p u c", p=P, u=NB)

    with tile.TileContext(nc) as tc:
        with (
            tc.tile_pool(name="ltp", bufs=LT_BUFS) as ltp,
            tc.tile_pool(name="persist", bufs=1) as persist,
            tc.tile_pool(name="stat", bufs=2 * NG_S) as statp,
            tc.tile_pool(name="work", bufs=2) as work,
            tc.tile_pool(name="acc", bufs=4, space="PSUM") as accp,
            tc.tile_pool(name="tpp", bufs=4, space="PSUM") as tpp,
            tc.tile_pool(name="dram", bufs=1, space="DRAM") as dram,
        ):
            # ---- persistent small tensors ----
            w_sb = persist.tile([P, C_OUT], mm_dt)
            nc.scalar.dma_start(w_sb[:], wf.ap())
            w4_sb = persist.tile([C_IN, C_OUT], mm_dt)
            nc.scalar.dma_start(w4_sb[:], w4.ap())
            bias_sb = persist.tile([1, C_OUT], f32)
            nc.scalar.dma_start(bias_sb[:], bias_in.ap())
            ones_sb = persist.tile([1, 512], f32)
            nc.vector.memset(ones_sb[:], 1.0)
            id_sb = persist.tile([P, C_IN], mm_dt)
            nc.scalar.dma_start(id_sb[:], id128.ap())

            # T_0..T_3 stacked at partition bases {0,32,64,96} of one tile
            # (rows 16-31 of each block must be ZERO: the fused einsum
            # contracts all 128 partitions against zero-padded weights).
            t_blk = persist.tile([P, VLOC], mm_dt)
            t4_sb = persist.tile([C_IN, VLOC], mm_dt)
            nc.vector.memset(t_blk[:], 0.0)
            nc.scalar.dma_start(t_blk[0:C_IN, :], xc.ap())

            def t_ap(k):
                if k == K_CHEB - 1:
                    return t4_sb[:]
                return t_blk[32 * k:32 * k + C_IN, :]

            # stationary tensors (x^T, then each gathered T_k^T) live in
            # rank-aligned 1536-row groups, row-interleaved so partition p
            # reads rows [g*1536 + 12p, +12) — one 384 B chunk.
            def load_stat(src_r, tag_k):
                tiles = []
                for g in range(NG_S):
                    s = statp.tile([P, NB_S * C_IN], mm_dt,
                                   name=f"st{tag_k}_{g}", tag="stat")
                    nc.scalar.dma_start(
                        s[:].rearrange("p (j c) -> p j c", j=NB_S), src_r[g])
                    tiles.append(s)
                return tiles

            xt_r = xt.ap().rearrange("(g p j) c -> g p j c", p=P, j=NB_S)
            sk_tiles = load_stat(xt_r, 0)

            # tiny warm-up AllGathers: pay the ~70us first-collective cost
            # while step 1 streams, not on the critical path
            wu_sb = work.tile([P, C_IN], mm_dt, name="wu_sb", tag="wu")
            nc.vector.memset(wu_sb[:], 0.0)
            wu_in = dram.tile([P, C_IN], mm_dt, name="wu_in0")
            wu_out = dram.tile([P * N_CORES, C_IN], mm_dt, name="wu_out0")
            nc.scalar.dma_start(wu_in[:], wu_sb[:])
            nc.gpsimd.collective_compute(
                "AllGather",
                mybir.AluOpType.bypass,
                replica_groups=[list(range(N_CORES))],
                ins=[wu_in.opt()],
                outs=[wu_out.opt()],
            )

            # resident lt: LAST RES_T vc-tiles.  Loaded just-in-time during
            # step 1 on the same sync HWDGE ring as the stream (issue order
            # = consumption order; HBM is the shared bottleneck), then
            # reused DMA-free by steps 2-4.  The scalar ring stays light so
            # the one-time CC setup isn't starved.
            rs_g = [persist.tile([P, NB * VLOC], mm_dt, name=f"rs{i}")
                    for i in range(res_g)]

            def size_matched_warmup(src, tag):
                # re-syncs the ranks and warms the exact transfer shape of
                # the real per-step all-gathers
                wsb = work.tile([P, NB_S * C_IN], mm_dt,
                                name=f"wu_{tag}", tag="scs")
                nc.vector.tensor_copy(wsb[:], src[:, :NB_S * C_IN])
                win = dram.tile([VLOC, C_IN], mm_dt, name=f"wuin_{tag}")
                wout = dram.tile([V, C_IN], mm_dt, name=f"wuout_{tag}")
                nc.scalar.dma_start(
                    win.rearrange("(p j) c -> p j c", p=P),
                    wsb[:].rearrange("p (j c) -> p j c", j=NB_S))
                nc.gpsimd.collective_compute(
                    "AllGather",
                    mybir.AluOpType.bypass,
                    replica_groups=[list(range(N_CORES))],
                    ins=[win.opt()],
                    outs=[wout.opt()],
                )

            # group order: interleave streamed (S, DMA 4.7us / PE 2.6us)
            # and resident (R, PE-only 2.6us) so neither engine starves in
            # the PE-bound steps 2-4; the two surplus S's sit mid-step, and
            # the step ends on R's (DMA idles there -> prefetches the next
            # step through the boundary).
            n_s = NG - res_g
            s_list = list(range(n_s))
            r_list = list(range(n_s, NG))
            g_order = []
            si = ri = 0
            for p_i in range(res_g):
                g_order.append(s_list[si]); si += 1
                if p_i in (3, 7) and si < n_s:
                    g_order.append(s_list[si]); si += 1
                g_order.append(r_list[ri]); ri += 1
            g_order.extend(s_list[si:])
            # keep the final slot resident: swap any trailing S inward
            while g_order[-1] < n_s:
                g_order.insert(len(g_order) - 2, g_order.pop())

            ein = None
            for k in range(1, K_CHEB):
                acc = [accp.tile([C_IN, 512], f32, name=f"acc{k}_{ch}",
                                 tag="acc") for ch in range(N_CH)]
                if k == K_CHEB - 1:
                    # T_0..T_3 einsum term: one matmul per chunk contracting
                    # all 128 t_blk partitions (zero-padded rows match wf's
                    # zeros).  Runs during step 4 on the idle transpose-pool
                    # banks; the T_4 term accumulates on top in the tail.
                    ein = [tpp.tile([C_OUT, 512], f32, name=f"ein{ch}",
                                    tag="tp") for ch in range(N_CH)]
                    for ch in range(N_CH):
                        nc.tensor.matmul(
                            ein[ch][:], lhsT=bias_sb[:], rhs=ones_sb[:],
                            start=True, stop=False)
                        nc.tensor.matmul(
                            ein[ch][:], lhsT=w_sb[:],
                            rhs=t_blk[:, ch * 512:(ch + 1) * 512],
                            start=False, stop=False)
                for gi, g in enumerate(g_order):
                    if g >= NG - res_g:
                        src = rs_g[g - (NG - res_g)]
                        if k == 1:
                            nc.sync.dma_start(
                                src[:].rearrange("p (u c) -> p u c", u=NB),
                                lt_r[g])
                        base = 0
                    else:
                        src = ltp.tile([P, NB * VLOC], mm_dt,
                                       name=f"lt{k}_{g}", tag="lt")
                        nc.sync.dma_start(
                            src[:].rearrange("p (u c) -> p u c", u=NB),
                            lt_r[g])
                        base = 0
                    # final group runs chunk-major so psum chunks drain
                    # staggered and the boundary DVE/transposes overlap the
                    # remaining matmuls
                    if gi == NG - 1:
                        uch = [(u, ch) for ch in range(N_CH)
                               for u in range(NB)]
                    else:
                        uch = [(u, ch) for u in range(NB)
                               for ch in range(N_CH)]
                    for u, ch in uch:
                        j = g * NB + u
                        st = sk_tiles[j // NB_S]
                        us = j % NB_S
                        nc.tensor.matmul(
                            acc[ch][:],
                            lhsT=st[:, us * C_IN:(us + 1) * C_IN],
                            rhs=src[:, base + u * VLOC + ch * 512:
                                    base + u * VLOC + (ch + 1) * 512],
                            start=(gi == 0 and u == 0),
                            stop=(gi == NG - 1 and u == NB - 1),
                        )
                    if k == 1 and g == 7:
                        # one size-matched warm-up AG pinned inside step 1
                        # so the real boundary AGs run at the warm floor
                        size_matched_warmup(src, f"w{g}")

                # ---- boundary, pipelined per 512-chunk:
                # T_k = 2*psum - T_{k-2} (step 1: copy), transpose, stage ----
                if k < K_CHEB - 1:
                    cc_in = dram.tile([VLOC, C_IN], mm_dt, name=f"ccin{k}")
                    cc_in_r = cc_in.rearrange("(p j) c -> p j c", p=P)
                    sc_stage = work.tile([P, (VLOC // P) * C_IN], mm_dt,
                                         name=f"scs{k}", tag="scs")
                tb = 32 * k
                for ch in range(N_CH):
                    sl = slice(ch * 512, (ch + 1) * 512)
                    if k == 1:
                        nc.vector.tensor_copy(t_ap(k)[:, sl], acc[ch][:])
                    else:
                        nc.vector.scalar_tensor_tensor(
                            t_ap(k)[:, sl], acc[ch][:], 2.0,
                            t_ap(k - 2)[:, sl],
                            mybir.AluOpType.mult, mybir.AluOpType.subtract)
                    if k < K_CHEB - 1:
                        for j2 in range(ch * 4, ch * 4 + 4):
                            tp_ps = tpp.tile([P, C_IN], mm_dt,
                                             name=f"tp{k}_{j2}", tag="tp")
                            nc.tensor.transpose(
                                tp_ps[:],
                                t_blk[tb:tb + C_IN, j2 * P:(j2 + 1) * P],
                                id_sb[tb:tb + C_IN, :],
                                tile_position=(tb, 0) if tb == 96 else None)
                            nc.vector.tensor_copy(
                                sc_stage[:, j2 * C_IN:(j2 + 1) * C_IN],
                                tp_ps[:])
                        nc.scalar.dma_start(
                            cc_in_r[:, ch * 4:ch * 4 + 4, :],
                            sc_stage[:, ch * 4 * C_IN:(ch * 4 + 4) * C_IN]
                            .rearrange("p (j c) -> p j c", j=4))

                if k < K_CHEB - 1:
                    cc_out = dram.tile([V, C_IN], mm_dt, name=f"ccout{k}")
                    nc.gpsimd.collective_compute(
                        "AllGather",
                        mybir.AluOpType.bypass,
                        replica_groups=[list(range(N_CORES))],
                        ins=[cc_in.opt()],
                        outs=[cc_out.opt()],
                    )
                    cc_r = cc_out.rearrange("(g p j) c -> g p j c",
                                            p=P, j=NB_S)
                    sk_tiles = load_stat(cc_r, k)

            # ---- tail: accumulate the T_4 term per chunk, store
            # straight from PSUM (bias already in the accumulation)
            for ch in range(N_CH):
                sl = slice(ch * 512, (ch + 1) * 512)
                nc.tensor.matmul(ein[ch][:], lhsT=w4_sb[:], rhs=t4_sb[:, sl],
                                 start=False, stop=True)
                res = work.tile([C_OUT, 512], f32, name=f"res{ch}",
                                tag="res")
                nc.vector.tensor_copy(res[:], ein[ch][:])
                nc.scalar.dma_start(out.ap()[:, sl], res[:])

    nc.compile()
    return nc


def _interleave_rows(a, nb):
    """Within each nb*128-row group, reorder rows so row g*G+nb*p+u holds
    original row g*G+u*128+p (one contiguous per-partition read)."""
    ng = a.shape[0] // (P * nb)
    return np.ascontiguousarray(
        a.reshape(ng, nb, P, a.shape[1]).transpose(0, 2, 1, 3)
        .reshape(a.shape))


def _prep_inputs(x, L, weights, bias, cfg: str):
    np_dt = ml_dtypes.bfloat16 if cfg == "bf16" else np.float32
    x = np.asarray(x, dtype=np.float32)
    L = np.asarray(L, dtype=np.float32)
    weights = np.asarray(weights, dtype=np.float32)
    bias = np.asarray(bias, dtype=np.float32)

    Lt = np.ascontiguousarray(L.T).astype(np_dt)          # (V, V)
    xt = _interleave_rows(
        np.ascontiguousarray(x.T).astype(np_dt), NB_S)    # (V, C_IN)

    wf = np.zeros((P, C_OUT), dtype=np_dt)
    for k in range(K_CHEB - 1):
        wf[32 * k:32 * k + C_IN, :] = weights[k]
    w4 = np.ascontiguousarray(weights[K_CHEB - 1]).astype(np_dt)
    b_ = np.ascontiguousarray(bias.reshape(1, C_OUT))
    id128 = np.zeros((P, C_IN), dtype=np_dt)
    for p in range(P):
        if p % 32 < C_IN:
            id128[p, p % 32] = 1.0

    in_maps = []
    for d in range(N_CORES):
        cols = slice(d * VLOC, (d + 1) * VLOC)
        in_maps.append({
            "lt": _interleave_rows(np.ascontiguousarray(Lt[:, cols]), NB),
            "xt": xt,
            "xc": np.ascontiguousarray(x[:, cols]).astype(np_dt),
            "wf": wf,
            "w4": w4,
            "bias_in": b_,
            "id128": id128,
        })
    return in_maps


def run(x, L, weights, bias, cfg: str = "bf16", trace: bool = False,
        trace_cores=None):
    if cfg not in _CACHE:
        _CACHE[cfg] = _build(cfg)
    nc = _CACHE[cfg]
    in_maps = _prep_inputs(x, L, weights, bias, cfg)
    kw = {}
    if trace_cores is not None:
        kw["trace_cores"] = trace_cores
    res = bass_utils.run_bass_kernel_spmd(
        nc, in_maps, core_ids=list(range(N_CORES)), trace=trace, **kw)
    out = np.concatenate([res.results[d]["out"] for d in range(N_CORES)],
                         axis=1)
    return out.astype(np.float32), res


def kernel(x, L, weights, bias):
    out, _ = run(x, L, weights, bias, cfg="bf16")
    return out
